# revision 1
# baseline (speedup 1.0000x reference)
"""BitLinear forward on 8 TRN2 NeuronCores — data-parallel over tokens.

Math: reference computes
    gamma_w = mean|W| + eps;  bw = clip(round(W/gamma_w), -1, 1)
    xn = LayerNorm(x);  gamma = max|xn|;  xq = clip(xn*QB/gamma, +-(QB-eps))
    y  = (xq @ bw.T) * (gamma*beta/QB),  beta = max_d sum_o |W[o,d]|
The gamma factor cancels algebraically (clip only nudges the max element
by 1e-5/127 ~ 8e-8 relative), so on device we compute
    y = (LayerNorm(x) @ bw.T) * beta
with NO cross-core collective (verified 6.6e-11 rel err vs reference in
f64; collectives also put the chip in the P0 power state, downclocking
the PE 2.4->2.0 GHz).  Ternary weights use a sign LUT split across
ScalarE and VectorE: stored bwts = sign(W-thr) + (-2)*[W<=-thr]
= bw2 - 1 in {1,-1,-3} (thr = gamma_w/2, bw2 = 2*clip(round(W/gamma_w)));
the uniform -1 offset cancels exactly through the rank-1 mu*colsum
correction (the pipeline is linear in the weights), and the factor 2
is folded into the beta epilogue scale.

LayerNorm is folded into the matmul epilogue so the main matmul can
start while inputs are still streaming in:
    y[t,o] = rstd[t]*beta' * ( sum_d xb[d,t]*bw2[d,o] - mu[t]*colsum[o] )
The -mu*colsum rank-1 term is ONE extra matmul accumulated into the same
PSUM group (lhsT = rows 0-1 = -mu, rest zero; rhs rows = colsum+2048 and
the exact constant -2048, so bf16 never rounds the ~-2048 colsum values),
and rstd[t]*beta' is a per-token column scalar applied by the ScalarE
PSUM->SBUF epilogue copy.

Layout trick: host passes x and W pre-transposed (contraction dim d on
partitions), so both matmul operands and the output are in natural
layouts and the kernel needs zero on-device transposes.  Per-token
LN statistics over d(=partitions) come from an all-ones stationary
matmul, which also broadcasts results to all partitions for free; the
token-indexed scalars are columnized via a tiny DRAM gather round-trip.
"""

import os
import sys

import numpy as np

for _p in ("/opt/trn_rl_repo", "/root/.axon_site/_ro/trn_rl_repo"):
    if os.path.isdir(_p) and _p not in sys.path:
        sys.path.append(_p)

from concourse import bacc, bass_isa, mybir, tile  # noqa: E402
from concourse.bass_utils import run_bass_kernel_spmd  # noqa: E402

P = 128
D = 2048  # contraction (hidden) dim
O = 2048  # output dim
N_CORES = 8
N_TOK = 4 * 4096
TOK = N_TOK // N_CORES  # tokens per core
KT = D // P  # 16 contraction tiles
MT = TOK // P  # 16 token tiles per core
CH = 512  # psum free chunk (one bank of f32)
NCH = O // CH
EPS = 1e-5
F32 = mybir.dt.float32
BF16 = mybir.dt.bfloat16


def build_nc():
    nc = bacc.Bacc(None, target_bir_lowering=False, debug=False)
    xt = nc.declare_dram_parameter("xt", [D, TOK], F32, isOutput=False)
    fwt = nc.declare_dram_parameter("fwt", [D, O], F32, isOutput=False)
    y = nc.declare_dram_parameter("y", [TOK, O], F32, isOutput=True)

    Alu = mybir.AluOpType
    Act = mybir.ActivationFunctionType
    Ax = mybir.AxisListType

    with tile.TileContext(nc) as tc:
        with (
            tc.tile_pool(name="const", bufs=1) as const,
            tc.tile_pool(name="wpool", bufs=2) as wpool,
            tc.tile_pool(name="bneg", bufs=2) as bnegp,
            tc.tile_pool(name="bw", bufs=KT) as bwp,
            tc.tile_pool(name="xpool", bufs=2) as xpool,
            tc.tile_pool(name="xb", bufs=KT) as xbp,
            tc.tile_pool(name="sq", bufs=2) as sqp,
            tc.tile_pool(name="stt", bufs=3) as stt,
            tc.tile_pool(name="rows", bufs=1) as rows,
            tc.tile_pool(name="ypool", bufs=2) as ypool,
            tc.tile_pool(name="dram", bufs=1, space="DRAM") as dpool,
            tc.tile_pool(name="psum", bufs=8, space="PSUM") as psum,
        ):
            ones_b = const.tile([P, P], BF16)
            nc.vector.memset(ones_b, 1.0)
            eps_t = const.tile([P, 1], F32)
            nc.vector.memset(eps_t, EPS)
            scal = const.tile([P, 8], F32)  # columns: scalar registry

            # ---- X ingest + LN stats (colsums via ones-matmul) ---------
            xbs = []
            ps_mu = [psum.tile([P, CH], F32, tag="ps", name=f"ps_mu{c}") for c in range(NCH)]
            ps_sq = [psum.tile([P, CH], F32, tag="ps", name=f"ps_sq{c}") for c in range(NCH)]
            wsum = const.tile([P, KT], F32)
            for k in range(KT):
                xk = xpool.tile([P, TOK], F32, tag="x")
                nc.sync.dma_start(xk, xt[P * k : P * (k + 1), :])
                wa = wpool.tile([P, O], F32, tag="w")
                nc.sync.dma_start(wa, fwt[P * k : P * (k + 1), :])
                nc.scalar.activation(
                    wa, wa, Act.Abs, accum_out=wsum[:, k : k + 1]
                )
                xb = xbp.tile([P, TOK], BF16, tag="xb")
                nc.vector.tensor_copy(out=xb, in_=xk)
                xbs.append(xb)
                first, last = k == 0, k == KT - 1
                for c in range(NCH):
                    sl = slice(CH * c, CH * (c + 1))
                    xsq = sqp.tile([P, CH], BF16, tag="xsq")
                    if c < NCH // 2:
                        nc.scalar.activation(xsq, xb[:, sl], Act.Square)
                    else:
                        nc.vector.tensor_tensor(
                            out=xsq, in0=xb[:, sl], in1=xb[:, sl], op=Alu.mult
                        )
                    nc.tensor.matmul(
                        ps_mu[c], ones_b, xb[:, sl], start=first, stop=last
                    )
                    nc.tensor.matmul(ps_sq[c], ones_b, xsq, start=first, stop=last)

            # ---- LN stats finalize:
            #   negmu row0 = -mu (rest 0), rb_row = rstd (beta folded later)
            negmu = rows.tile([P, TOK], BF16, tag="negmu")
            nc.vector.memset(negmu, 0.0)
            rb_row = rows.tile([1, TOK], F32, tag="rb_row")
            for c in range(NCH):
                sl = slice(CH * c, CH * (c + 1))
                mu_c = stt.tile([P, CH], F32, tag="stt")
                nc.scalar.mul(mu_c, ps_mu[c], 1.0 / D)
                var_c = stt.tile([P, CH], F32, tag="stt")
                nc.scalar.mul(var_c, ps_sq[c], 1.0 / D)  # E[x^2]
                nc.scalar.activation(
                    negmu[0:2, sl], mu_c[0:2, :], Act.Copy, bias=0.0, scale=-1.0
                )
                musq = stt.tile([P, CH], F32, tag="stt")
                nc.scalar.activation(musq, mu_c, Act.Square)
                nc.vector.tensor_tensor(
                    out=var_c, in0=var_c, in1=musq, op=Alu.subtract
                )
                nc.scalar.activation(var_c, var_c, Act.Sqrt, bias=eps_t)
                rstd_c = stt.tile([P, CH], F32, tag="stt")
                nc.vector.reciprocal(rstd_c, var_c)
                nc.vector.tensor_copy(out=rb_row[0:1, sl], in_=rstd_c[0:1, :])

            # columnize rb_row: [1, TOK] -> [P, MT] via DRAM gather ------
            rb_dram = dpool.tile([TOK], F32)
            nc.sync.dma_start(rb_dram[None, :], rb_row)
            rb_col = const.tile([P, MT], F32)
            with nc.allow_non_contiguous_dma(reason="2048x4B one-time gather"):
                nc.sync.dma_start(rb_col, rb_dram.rearrange("(m p) -> p m", p=P))


            row_tot = scal[:, 0:1]  # per-partition total of |W|
            nc.vector.tensor_reduce(row_tot, wsum, axis=Ax.X, op=Alu.add)
            beta_pp = scal[:, 1:2]  # per-partition max row-sum
            nc.vector.tensor_reduce(beta_pp, wsum, axis=Ax.X, op=Alu.max)
            tot_b = scal[:, 2:3]
            nc.gpsimd.partition_all_reduce(
                tot_b, row_tot, channels=P, reduce_op=bass_isa.ReduceOp.add
            )
            beta_b = scal[:, 3:4]
            nc.gpsimd.partition_all_reduce(
                beta_b, beta_pp, channels=P, reduce_op=bass_isa.ReduceOp.max
            )
            # thr = 0.5*gamma_w = 0.5*(tot/(D*O) + EPS)
            thr = scal[:, 4:5]
            nc.scalar.activation(
                thr, tot_b, Act.Copy, bias=0.5 * EPS, scale=0.5 / (D * O)
            )
            nthr = scal[:, 5:6]
            nc.scalar.activation(
                nthr, tot_b, Act.Copy, bias=-0.5 * EPS, scale=-0.5 / (D * O)
            )
            beta_h = scal[:, 6:7]  # beta/2 (bw carries a factor of 2)
            nc.scalar.activation(beta_h, beta_b, Act.Copy, bias=0.0, scale=0.5)


            rbb_col = const.tile([P, MT], F32)  # rstd[t] * beta/2, columnized
            nc.vector.tensor_scalar(
                out=rbb_col, in0=rb_col, scalar1=beta_h, scalar2=None,
                op0=Alu.mult,
            )
            # ---- W pass B: ternarize via sign LUT ----------------------
            # bw2 = sign(W - thr) + sign(W + thr) in {-2, 0, +2}
            ps_cs = [psum.tile([P, CH], F32, tag="ps", name=f"ps_cs{c}") for c in range(NCH)]
            bwts = []
            for i in range(KT):
                wb = wpool.tile([P, O], F32, tag="w")
                nc.sync.dma_start(wb, fwt[P * i : P * (i + 1), :])
                bw = bwp.tile([P, O], BF16, tag="bw")
                nc.scalar.activation(bw, wb, Act.Sign, bias=nthr)
                bneg = bnegp.tile([P, O], BF16, tag="bneg")
                nc.vector.tensor_scalar(
                    out=bneg, in0=wb, scalar1=nthr, scalar2=-2.0,
                    op0=Alu.is_le, op1=Alu.mult,
                )
                # stored weights = bw2 - 1 in {1,-1,-3}; the uniform -1
                # offset cancels exactly through the -mu*colsum correction
                nc.vector.tensor_tensor(out=bw, in0=bw, in1=bneg, op=Alu.add)
                bwts.append(bw)
                for c in range(NCH):
                    nc.tensor.matmul(
                        ps_cs[c], ones_b, bw[:, CH * c : CH * (c + 1)],
                        start=(i == 0), stop=(i == KT - 1),
                    )

            crep = []
            for c in range(NCH):
                ct = rows.tile([P, CH], BF16, tag=f"crep{c}")
                nc.vector.memset(ct, 0.0)
                nc.vector.memset(ct[0:2, :], -2048.0)
                nc.scalar.activation(
                    ct[0:1, :], ps_cs[c][0:1, :], Act.Copy, bias=2048.0
                )
                crep.append(ct)


            # ---- main matmul + fused LN epilogue -----------------------
            for m in range(MT):
                pys = [psum.tile([P, CH], F32, tag="ps", name=f"py{m}_{c}") for c in range(NCH)]
                for k in range(KT):
                    lhs = xbs[k][:, P * m : P * (m + 1)]
                    first = k == 0
                    for c in range(NCH):
                        nc.tensor.matmul(
                            pys[c],
                            lhs,
                            bwts[k][:, CH * c : CH * (c + 1)],
                            start=first,
                            stop=False,
                        )
                # rank-1 correction: psum += (-mu[t]) * colsum[o]
                nmslice = negmu[:, P * m : P * (m + 1)]
                for c in range(NCH):
                    nc.tensor.matmul(pys[c], nmslice, crep[c], start=False, stop=True)
                for c in range(NCH):
                    ysb = ypool.tile([P, CH], F32, tag="y")
                    nc.scalar.mul(ysb, pys[c], rbb_col[:, m : m + 1])
                    nc.sync.dma_start(
                        y[P * m : P * (m + 1), CH * c : CH * (c + 1)], ysb
                    )

    nc.compile()
    return nc


_NC_CACHE = None


def _get_nc():
    global _NC_CACHE
    if _NC_CACHE is None:
        _NC_CACHE = build_nc()
    return _NC_CACHE


def _prep_in_maps(x, fweight):
    x2 = np.ascontiguousarray(x, dtype=np.float32).reshape(N_TOK, D)
    fwt = np.ascontiguousarray(np.asarray(fweight, dtype=np.float32).T)
    in_maps = []
    for c in range(N_CORES):
        xs = np.ascontiguousarray(x2[c * TOK : (c + 1) * TOK, :].T)
        in_maps.append({"xt": xs, "fwt": fwt})
    return in_maps


def run_spmd(x, fweight, **kw):
    nc = _get_nc()
    in_maps = _prep_in_maps(x, fweight)
    return run_bass_kernel_spmd(nc, in_maps, core_ids=list(range(N_CORES)), **kw)


def kernel(x, fweight):
    res = run_spmd(x, fweight)
    y = np.concatenate([res.results[c]["y"] for c in range(N_CORES)], axis=0)
    return y.reshape(4, 4096, O)


if __name__ == "__main__":
    xx = np.random.randn(4, 4096, D).astype(np.float32)
    ww = np.random.uniform(-1 / np.sqrt(D), 1 / np.sqrt(D), (O, D)).astype(np.float32)
    out = kernel(xx, ww)
    print("out", out.shape, out.dtype, float(np.abs(out).mean()))



# revision 4
# speedup vs baseline: 1.2064x; 1.2064x over previous
"""BitLinear forward on 8 TRN2 NeuronCores — data-parallel over tokens.

v2: restructured for PE occupancy.  The baseline spent ~200us of its
~480us span in a DMA-bound prelude (x+W ingest, stats matmuls trickling
at cold HAM clocks, a serial W re-read for ternarize) before a 271us
PE-bound main matmul.  v2 compresses the prelude to ~60us:

  - W streams in FIRST (gamma_w gates everything); most W f32 tiles stay
    resident in SBUF so ternarize needs only a small re-DMA tail.
  - Ternarize (scalar Sign + vector is_le + gpsimd add), the colsum
    matmul, and the FIRST main-matmul group are interleaved per k-tile,
    so the PE starts real work the moment bw[0] exists.
  - x arrives as bf16 from the host (device cast was bf16 anyway) in two
    layouts: token-blocked [m][d][k*tau] for the matmul stationary
    (contiguous 4KiB/partition lines, only 0.5MiB needed before the main
    matmul starts) and natural [tok, d] rows for LN stats, which become
    free-dim reduces on vector/scalar — no stats matmuls, no PSUM
    pressure, and the per-token stats land directly columnized (no DRAM
    gather round-trip).
  - The rank-1 -mu*colsum LN correction moved off the PE into the
    epilogue: y = rstd*beta/2*psum - (mu*rstd*beta/2)*colsum, which is
    exact for stored weights s = 2*bw + c for ANY constant c (the c
    terms cancel), so ternary weights are stored as {1,-1,-3} with no
    correction matmul and no bf16-exactness hacks.

Math (gamma cancels as in the baseline; verified 6.6e-11 in f64):
    gamma_w = mean|W| + eps;  bw = clip(round(W/gamma_w), -1, 1)
    y = (LayerNorm(x) @ bw.T) * beta,   beta = max_d sum_o |W^T[d,o]|
No cross-core collective (collectives also downclock the PE 2.4->2.0).
"""

import os
import sys

import numpy as np

for _p in ("/opt/trn_rl_repo", "/root/.axon_site/_ro/trn_rl_repo"):
    if os.path.isdir(_p) and _p not in sys.path:
        sys.path.append(_p)

from concourse import bacc, bass_isa, mybir, tile  # noqa: E402
from concourse.bass_utils import run_bass_kernel_spmd  # noqa: E402

import ml_dtypes  # noqa: E402

P = 128
D = 2048  # contraction (hidden) dim
O = 2048  # output dim
N_CORES = 8
N_TOK = 4 * 4096
TOK = N_TOK // N_CORES  # tokens per core
KT = D // P  # 16 contraction tiles
MT = TOK // P  # 16 token tiles per core
CH = 512  # psum free chunk (one bank of f32)
NCH = O // CH
WRES = 6  # W f32 tiles resident in SBUF (rest re-DMA'd for ternarize)
EPS = 1e-5
F32 = mybir.dt.float32
BF16 = mybir.dt.bfloat16


def build_nc():
    nc = bacc.Bacc(None, target_bir_lowering=False, debug=False)
    # token-blocked x for the matmul: xm[m][p][k*128+tau] = x[t=128m+tau, d=128k+p]
    xm_d = nc.declare_dram_parameter("xm", [MT * P, KT * P], BF16, isOutput=False)
    # natural token-rows x for LN stats
    xr_d = nc.declare_dram_parameter("xr", [TOK, D], BF16, isOutput=False)
    fwt = nc.declare_dram_parameter("fwt", [D, O], F32, isOutput=False)
    y = nc.declare_dram_parameter("y", [TOK, O], F32, isOutput=True)

    Alu = mybir.AluOpType
    Act = mybir.ActivationFunctionType
    Ax = mybir.AxisListType

    with tile.TileContext(nc) as tc:
        with (
            tc.tile_pool(name="const", bufs=1) as const,
            tc.tile_pool(name="wres", bufs=WRES) as wres,
            tc.tile_pool(name="wstr", bufs=2) as wstr,
            tc.tile_pool(name="scr", bufs=2) as scr,
            tc.tile_pool(name="bneg", bufs=2) as bnegp,
            tc.tile_pool(name="bw", bufs=KT) as bwp,
            tc.tile_pool(name="xm", bufs=3) as xmp,
            tc.tile_pool(name="xrow", bufs=4) as xrp,
            tc.tile_pool(name="cs2", bufs=NCH) as cs2p,
            tc.tile_pool(name="ysb", bufs=3) as ypool,
            tc.tile_pool(name="corr", bufs=2) as corrp,
            tc.tile_pool(name="psum", bufs=8, space="PSUM") as psum,
        ):
            ones_b = const.tile([P, P], BF16)
            nc.vector.memset(ones_b, 1.0)
            eps_t = const.tile([P, 1], F32)
            nc.vector.memset(eps_t, EPS)
            scal = const.tile([P, 8], F32)  # scalar registry (columns)
            wsum = const.tile([P, KT], F32)  # per-d-row sum of |W| per tile
            # per-token stats, columnized: [P, MT]
            sx_c = const.tile([P, MT], F32)
            sq_c = const.tile([P, MT], F32)
            rbb_c = const.tile([P, MT], F32)  # rstd * beta/2
            q_c = const.tile([P, MT], F32)  # mu * rstd * beta/2

            # ---- phase A: W ingest + |W| row sums ----------------------
            wtiles = []
            for k in range(KT):
                pool = wres if k < WRES else wstr
                wk = pool.tile([P, O], F32, tag="wr" if k < WRES else "ws")
                nc.sync.dma_start(wk, fwt[P * k : P * (k + 1), :])
                ab = scr.tile([P, O], BF16, tag="scr")
                nc.scalar.activation(
                    ab, wk, Act.Abs, accum_out=wsum[:, k : k + 1]
                )
                wtiles.append(wk if k < WRES else None)

            # x DMAs for m0 + stats chunk 0 queue right behind W on sync
            xm0 = xmp.tile([P, KT * P], BF16, tag="xm")
            nc.sync.dma_start(xm0, xm_d[0:P, :])
            xr0 = []
            for s in range(4):
                xr = xrp.tile([P, D], BF16, tag="xr")
                nc.sync.dma_start(xr, xr_d[P * s : P * (s + 1), :])
                xr0.append(xr)
            # W re-DMA tail for ternarize (paced by wstr pool reuse)
            wb_tail = {}
            for k in range(WRES, KT):
                wk = wstr.tile([P, O], F32, tag="ws")
                nc.sync.dma_start(wk, fwt[P * k : P * (k + 1), :])
                wb_tail[k] = wk

            # ---- gamma_w / beta scalars --------------------------------
            row_tot = scal[:, 0:1]
            nc.vector.tensor_reduce(row_tot, wsum, axis=Ax.X, op=Alu.add)
            beta_pp = scal[:, 1:2]
            nc.vector.tensor_reduce(beta_pp, wsum, axis=Ax.X, op=Alu.max)
            tot_b = scal[:, 2:3]
            nc.gpsimd.partition_all_reduce(
                tot_b, row_tot, channels=P, reduce_op=bass_isa.ReduceOp.add
            )
            beta_b = scal[:, 3:4]
            nc.gpsimd.partition_all_reduce(
                beta_b, beta_pp, channels=P, reduce_op=bass_isa.ReduceOp.max
            )
            # thr = 0.5*gamma_w = 0.5*(tot/(D*O) + EPS)
            nthr = scal[:, 5:6]
            nc.scalar.activation(
                nthr, tot_b, Act.Copy, bias=-0.5 * EPS, scale=-0.5 / (D * O)
            )
            beta_h = scal[:, 6:7]  # beta/2 (stored weights carry factor 2)
            nc.scalar.activation(beta_h, beta_b, Act.Copy, bias=0.0, scale=0.5)

            # ---- stats chunk helper (vector/scalar, no PE) -------------
            def stats_chunk(c, xr_tiles):
                for s in range(4):
                    m = 4 * c + s
                    nc.vector.tensor_reduce(
                        sx_c[:, m : m + 1], xr_tiles[s], axis=Ax.X, op=Alu.add
                    )
                    sq = scr.tile([P, D], BF16, tag="scr")
                    nc.scalar.activation(
                        sq, xr_tiles[s], Act.Square,
                        accum_out=sq_c[:, m : m + 1],
                    )
                sl = slice(4 * c, 4 * c + 4)
                # finalize on [P,4] slices (tiny)
                mu_t = scr_small[c][:, 0:4]
                nc.scalar.activation(
                    mu_t, sx_c[:, sl], Act.Copy, bias=0.0, scale=1.0 / D
                )
                ex2 = scr_small[c][:, 4:8]
                nc.scalar.activation(
                    ex2, sq_c[:, sl], Act.Copy, bias=0.0, scale=1.0 / D
                )
                musq = scr_small[c][:, 8:12]
                nc.scalar.activation(musq, mu_t, Act.Square)
                var = scr_small[c][:, 12:16]
                nc.vector.tensor_tensor(out=var, in0=ex2, in1=musq, op=Alu.subtract)
                nc.scalar.activation(var, var, Act.Sqrt, bias=eps_t)
                rstd = scr_small[c][:, 16:20]
                nc.vector.reciprocal(rstd, var)
                nc.vector.tensor_scalar(
                    out=rbb_c[:, sl], in0=rstd, scalar1=beta_h, scalar2=None,
                    op0=Alu.mult,
                )
                qq = scr_small[c][:, 20:24]
                nc.vector.tensor_tensor(out=qq, in0=mu_t, in1=rstd, op=Alu.mult)
                nc.vector.tensor_scalar(
                    out=q_c[:, sl], in0=qq, scalar1=beta_h, scalar2=None,
                    op0=Alu.mult,
                )

            scr_small = [
                const.tile([P, 24], F32, tag=f"ss{c}", name=f"ss{c}")
                for c in range(4)
            ]

            # ---- interleaved: ternarize k + colsum MM + main m0 MMs ----
            ps_cs = [
                psum.tile([P, CH], F32, tag="ps", name=f"cs{c}") for c in range(NCH)
            ]
            py0 = [
                psum.tile([P, CH], F32, tag="ps", name=f"py0_{c}") for c in range(NCH)
            ]
            bwts = []
            for k in range(KT):
                wk = wtiles[k] if k < WRES else wb_tail[k]
                bw = bwp.tile([P, O], BF16, tag="bw")
                # sgn = Sign(W - thr) in {-1,+1}
                nc.scalar.activation(bw, wk, Act.Sign, bias=nthr)
                bneg = bnegp.tile([P, O], BF16, tag="bneg")
                nc.vector.tensor_scalar(
                    out=bneg, in0=wk, scalar1=nthr, scalar2=-2.0,
                    op0=Alu.is_le, op1=Alu.mult,
                )
                # stored s = sgn + bneg in {1,-1,-3} = 2*bw - 1 (c=-1 cancels
                # exactly through the mu*colsum epilogue correction)
                nc.gpsimd.tensor_tensor(out=bw, in0=bw, in1=bneg, op=Alu.add)
                bwts.append(bw)
                first, last = k == 0, k == KT - 1
                for c in range(NCH):
                    sl = slice(CH * c, CH * (c + 1))
                    nc.tensor.matmul(
                        ps_cs[c], ones_b, bw[:, sl], start=first, stop=last
                    )
                for c in range(NCH):
                    sl = slice(CH * c, CH * (c + 1))
                    nc.tensor.matmul(
                        py0[c], xm0[:, P * k : P * (k + 1)], bw[:, sl],
                        start=first, stop=last,
                    )
                if k == WRES:
                    # resident-tile ternarize done; engines idle while the
                    # W re-DMA tail streams -> do stats chunk 0 here
                    stats_chunk(0, xr0)

            # colsum -> SBUF f32 (broadcast over partitions already)
            cs2 = []
            for c in range(NCH):
                ct = cs2p.tile([P, CH], F32, tag="cs2")
                nc.vector.tensor_copy(out=ct, in_=ps_cs[c])
                cs2.append(ct)

            # ---- epilogue helper ---------------------------------------
            def epilogue(m, pys):
                for c in range(NCH):
                    ysb = ypool.tile([P, CH], F32, tag="y")
                    nc.scalar.mul(ysb, pys[c], rbb_c[:, m : m + 1])
                    corr = corrp.tile([P, CH], F32, tag="corr")
                    nc.vector.tensor_scalar(
                        out=corr, in0=cs2[c], scalar1=q_c[:, m : m + 1],
                        scalar2=None, op0=Alu.mult,
                    )
                    nc.vector.tensor_tensor(out=ysb, in0=ysb, in1=corr, op=Alu.subtract)
                    nc.gpsimd.dma_start(
                        y[P * m : P * (m + 1), CH * c : CH * (c + 1)], ysb
                    )

            epilogue(0, py0)

            # ---- main loop m = 1..15 -----------------------------------
            xr_pending = {}
            for m in range(1, MT):
                xmt = xmp.tile([P, KT * P], BF16, tag="xm")
                nc.sync.dma_start(xmt, xm_d[P * m : P * (m + 1), :])
                # stats chunks 1..3: DMA xrows early, compute when queued
                if m in (1, 5, 9):
                    cc = (m + 3) // 4
                    tiles = []
                    for s in range(4):
                        xr = xrp.tile([P, D], BF16, tag="xr")
                        nc.sync.dma_start(
                            xr, xr_d[P * (4 * cc + s) : P * (4 * cc + s + 1), :]
                        )
                        tiles.append(xr)
                    xr_pending[cc] = tiles
                pys = [
                    psum.tile([P, CH], F32, tag="ps", name=f"py{m}_{c2}")
                    for c2 in range(NCH)
                ]
                for k in range(KT):
                    first, last = k == 0, k == KT - 1
                    lhs = xmt[:, P * k : P * (k + 1)]
                    for c2 in range(NCH):
                        nc.tensor.matmul(
                            pys[c2], lhs, bwts[k][:, CH * c2 : CH * (c2 + 1)],
                            start=first, stop=last,
                        )
                if m in (2, 6, 10):
                    cc = (m + 2) // 4
                    stats_chunk(cc, xr_pending.pop(cc))
                epilogue(m, pys)

    nc.compile()
    return nc


_NC_CACHE = None


def _get_nc():
    global _NC_CACHE
    if _NC_CACHE is None:
        _NC_CACHE = build_nc()
    return _NC_CACHE


def _prep_in_maps(x, fweight):
    bf16 = ml_dtypes.bfloat16
    x2 = np.ascontiguousarray(x, dtype=np.float32).reshape(N_TOK, D)
    fwt = np.ascontiguousarray(np.asarray(fweight, dtype=np.float32).T)
    in_maps = []
    for c in range(N_CORES):
        xc = x2[c * TOK : (c + 1) * TOK, :]
        xr = np.ascontiguousarray(xc).astype(bf16)
        xmb = np.ascontiguousarray(
            xc.reshape(MT, P, KT, P).transpose(0, 3, 2, 1)
        ).astype(bf16).reshape(MT * P, KT * P)
        in_maps.append({"xm": xmb, "xr": xr, "fwt": fwt})
    return in_maps


def run_spmd(x, fweight, **kw):
    nc = _get_nc()
    in_maps = _prep_in_maps(x, fweight)
    return run_bass_kernel_spmd(nc, in_maps, core_ids=list(range(N_CORES)), **kw)


def kernel(x, fweight):
    res = run_spmd(x, fweight)
    y = np.concatenate([res.results[c]["y"] for c in range(N_CORES)], axis=0)
    return y.reshape(4, 4096, O)


if __name__ == "__main__":
    xx = np.random.randn(4, 4096, D).astype(np.float32)
    ww = np.random.uniform(-1 / np.sqrt(D), 1 / np.sqrt(D), (O, D)).astype(np.float32)
    out = kernel(xx, ww)
    print("out", out.shape, out.dtype, float(np.abs(out).mean()))


# revision 6
# speedup vs baseline: 1.3983x; 1.1590x over previous
"""BitLinear forward on 8 TRN2 NeuronCores — data-parallel over tokens.

v2: restructured for PE occupancy.  The baseline spent ~200us of its
~480us span in a DMA-bound prelude (x+W ingest, stats matmuls trickling
at cold HAM clocks, a serial W re-read for ternarize) before a 271us
PE-bound main matmul.  v2 compresses the prelude to ~60us:

  - W streams in FIRST (gamma_w gates everything); most W f32 tiles stay
    resident in SBUF so ternarize needs only a small re-DMA tail.
  - Ternarize (scalar Sign + vector is_le + gpsimd add), the colsum
    matmul, and the FIRST main-matmul group are interleaved per k-tile,
    so the PE starts real work the moment bw[0] exists.
  - x arrives as bf16 from the host (device cast was bf16 anyway) in two
    layouts: token-blocked [m][d][k*tau] for the matmul stationary
    (contiguous 4KiB/partition lines, only 0.5MiB needed before the main
    matmul starts) and natural [tok, d] rows for LN stats, which become
    free-dim reduces on vector/scalar — no stats matmuls, no PSUM
    pressure, and the per-token stats land directly columnized (no DRAM
    gather round-trip).
  - The rank-1 -mu*colsum LN correction moved off the PE into the
    epilogue: y = rstd*beta/2*psum - (mu*rstd*beta/2)*colsum, which is
    exact for stored weights s = 2*bw + c for ANY constant c (the c
    terms cancel), so ternary weights are stored as {1,-1,-3} with no
    correction matmul and no bf16-exactness hacks.

Math (gamma cancels as in the baseline; verified 6.6e-11 in f64):
    gamma_w = mean|W| + eps;  bw = clip(round(W/gamma_w), -1, 1)
    y = (LayerNorm(x) @ bw.T) * beta,   beta = max_d sum_o |W^T[d,o]|
No cross-core collective (collectives also downclock the PE 2.4->2.0).
"""

import os
import sys

import numpy as np

for _p in ("/opt/trn_rl_repo", "/root/.axon_site/_ro/trn_rl_repo"):
    if os.path.isdir(_p) and _p not in sys.path:
        sys.path.append(_p)

from concourse import bacc, bass_isa, mybir, tile  # noqa: E402
from concourse.bass_utils import run_bass_kernel_spmd  # noqa: E402

import ml_dtypes  # noqa: E402

P = 128
D = 2048  # contraction (hidden) dim
O = 2048  # output dim
N_CORES = 8
N_TOK = 4 * 4096
TOK = N_TOK // N_CORES  # tokens per core
KT = D // P  # 16 contraction tiles
MT = TOK // P  # 16 token tiles per core
CH = 512  # psum free chunk (one bank of f32)
NCH = O // CH
WRES = 6  # W f32 tiles resident in SBUF (rest re-DMA'd for ternarize)
EPS = 1e-5
F32 = mybir.dt.float32
BF16 = mybir.dt.bfloat16


def build_nc():
    nc = bacc.Bacc(None, target_bir_lowering=False, debug=False)
    # token-blocked x for the matmul: xm[m][p][k*128+tau] = x[t=128m+tau, d=128k+p]
    xm_d = nc.declare_dram_parameter("xm", [MT * P, KT * P], BF16, isOutput=False)
    # natural token-rows x for LN stats
    xr_d = nc.declare_dram_parameter("xr", [TOK, D], BF16, isOutput=False)
    fwt = nc.declare_dram_parameter("fwt", [D, O], F32, isOutput=False)
    y = nc.declare_dram_parameter("y", [TOK, O], F32, isOutput=True)

    Alu = mybir.AluOpType
    Act = mybir.ActivationFunctionType
    Ax = mybir.AxisListType

    with tile.TileContext(nc) as tc:
        with (
            tc.tile_pool(name="const", bufs=1) as const,
            tc.tile_pool(name="wres", bufs=WRES) as wres,
            tc.tile_pool(name="wstr", bufs=3) as wstr,
            tc.tile_pool(name="scr", bufs=2) as scr,
            tc.tile_pool(name="bneg", bufs=2) as bnegp,
            tc.tile_pool(name="bw", bufs=KT) as bwp,
            tc.tile_pool(name="xm", bufs=2) as xmp,
            tc.tile_pool(name="xrow", bufs=4) as xrp,
            tc.tile_pool(name="cs2", bufs=NCH) as cs2p,
            tc.tile_pool(name="ysb", bufs=3) as ypool,
            tc.tile_pool(name="yout", bufs=3) as youtp,
            tc.tile_pool(name="psum", bufs=8, space="PSUM") as psum,
        ):
            ones_b = const.tile([P, P], BF16)
            nc.vector.memset(ones_b, 1.0)
            eps_t = const.tile([P, 1], F32)
            nc.vector.memset(eps_t, EPS)
            scal = const.tile([P, 8], F32)  # scalar registry (columns)
            wsum = const.tile([P, KT], F32)  # per-d-row sum of |W| per tile
            # per-token stats, columnized: [P, MT]
            sx_c = const.tile([P, MT], F32)
            sq_c = const.tile([P, MT], F32)
            rbb_c = const.tile([P, MT], F32)  # rstd * beta/2
            q_c = const.tile([P, MT], F32)  # mu * rstd * beta/2

            # ---- phase A: W ingest + |W| row sums ----------------------
            wtiles = []
            for k in range(KT):
                pool = wres if k < WRES else wstr
                wk = pool.tile([P, O], F32, tag="wr" if k < WRES else "ws")
                nc.sync.dma_start(wk, fwt[P * k : P * (k + 1), :])
                ab = scr.tile([P, O], BF16, tag="scr")
                nc.scalar.activation(
                    ab, wk, Act.Abs, accum_out=wsum[:, k : k + 1]
                )
                wtiles.append(wk if k < WRES else None)

            # x DMAs for m0 + stats chunk 0 queue right behind W on sync
            xm0 = xmp.tile([P, KT * P], BF16, tag="xm")
            nc.sync.dma_start(xm0, xm_d[0:P, :])
            xr0 = []
            for s in range(4):
                xr = xrp.tile([P, D], BF16, tag="xr")
                nc.sync.dma_start(xr, xr_d[P * s : P * (s + 1), :])
                xr0.append(xr)
            # W re-DMA tail for ternarize (paced by wstr pool reuse)
            wb_tail = {}
            for k in range(WRES, KT):
                wk = wstr.tile([P, O], F32, tag="ws")
                nc.sync.dma_start(wk, fwt[P * k : P * (k + 1), :])
                wb_tail[k] = wk

            # ---- gamma_w / beta scalars --------------------------------
            row_tot = scal[:, 0:1]
            nc.vector.tensor_reduce(row_tot, wsum, axis=Ax.X, op=Alu.add)
            beta_pp = scal[:, 1:2]
            nc.vector.tensor_reduce(beta_pp, wsum, axis=Ax.X, op=Alu.max)
            tot_b = scal[:, 2:3]
            nc.gpsimd.partition_all_reduce(
                tot_b, row_tot, channels=P, reduce_op=bass_isa.ReduceOp.add
            )
            beta_b = scal[:, 3:4]
            nc.gpsimd.partition_all_reduce(
                beta_b, beta_pp, channels=P, reduce_op=bass_isa.ReduceOp.max
            )
            # thr = 0.5*gamma_w = 0.5*(tot/(D*O) + EPS)
            nthr = scal[:, 5:6]
            nc.scalar.activation(
                nthr, tot_b, Act.Copy, bias=-0.5 * EPS, scale=-0.5 / (D * O)
            )
            # epilogue is one fused op: y = (cs2 * q) + psum*rbb with
            # rbb = +beta/2 * rstd and q = -beta/2 * mu * rstd
            beta_hn = scal[:, 6:7]  # -beta/2
            nc.scalar.activation(beta_hn, beta_b, Act.Copy, bias=0.0, scale=-0.5)
            beta_hp = scal[:, 7:8]  # +beta/2
            nc.scalar.activation(beta_hp, beta_b, Act.Copy, bias=0.0, scale=0.5)

            # ---- stats chunk helper (vector/scalar, no PE) -------------
            def stats_chunk(c, xr_tiles):
                for s in range(4):
                    m = 4 * c + s
                    nc.vector.tensor_reduce(
                        sx_c[:, m : m + 1], xr_tiles[s], axis=Ax.X, op=Alu.add
                    )
                    sq = scr.tile([P, D], BF16, tag="scr")
                    nc.scalar.activation(
                        sq, xr_tiles[s], Act.Square,
                        accum_out=sq_c[:, m : m + 1],
                    )
                sl = slice(4 * c, 4 * c + 4)
                # finalize on [P,4] slices (tiny)
                mu_t = scr_small[c][:, 0:4]
                nc.scalar.activation(
                    mu_t, sx_c[:, sl], Act.Copy, bias=0.0, scale=1.0 / D
                )
                ex2 = scr_small[c][:, 4:8]
                nc.scalar.activation(
                    ex2, sq_c[:, sl], Act.Copy, bias=0.0, scale=1.0 / D
                )
                musq = scr_small[c][:, 8:12]
                nc.scalar.activation(musq, mu_t, Act.Square)
                var = scr_small[c][:, 12:16]
                nc.vector.tensor_tensor(out=var, in0=ex2, in1=musq, op=Alu.subtract)
                nc.scalar.activation(var, var, Act.Sqrt, bias=eps_t)
                rstd = scr_small[c][:, 16:20]
                nc.vector.reciprocal(rstd, var)
                nc.vector.tensor_scalar(
                    out=rbb_c[:, sl], in0=rstd, scalar1=beta_hp, scalar2=None,
                    op0=Alu.mult,
                )
                qq = scr_small[c][:, 20:24]
                nc.vector.tensor_tensor(out=qq, in0=mu_t, in1=rstd, op=Alu.mult)
                nc.vector.tensor_scalar(
                    out=q_c[:, sl], in0=qq, scalar1=beta_hn, scalar2=None,
                    op0=Alu.mult,
                )

            scr_small = [
                const.tile([P, 24], F32, tag=f"ss{c}", name=f"ss{c}")
                for c in range(4)
            ]

            # ---- interleaved: ternarize k + colsum MM + main m0 MMs ----
            ps_cs = [
                psum.tile([P, CH], F32, tag="ps", name=f"cs{c}") for c in range(NCH)
            ]
            py0 = [
                psum.tile([P, CH], F32, tag="ps", name=f"py0_{c}") for c in range(NCH)
            ]
            bwts = []
            for k in range(KT):
                wk = wtiles[k] if k < WRES else wb_tail[k]
                bw = bwp.tile([P, O], BF16, tag="bw")
                # sgn = Sign(W - thr) in {-1,+1}
                nc.scalar.activation(bw, wk, Act.Sign, bias=nthr)
                bneg = bnegp.tile([P, O], BF16, tag="bneg")
                nc.vector.tensor_scalar(
                    out=bneg, in0=wk, scalar1=nthr, scalar2=-2.0,
                    op0=Alu.is_le, op1=Alu.mult,
                )
                # stored s = sgn + bneg in {1,-1,-3} = 2*bw - 1; the uniform
                # -1 offset cancels through the mu*colsum epilogue correction
                nc.vector.tensor_tensor(out=bw, in0=bw, in1=bneg, op=Alu.add)
                bwts.append(bw)
                first, last = k == 0, k == KT - 1
                for c in range(NCH):
                    sl = slice(CH * c, CH * (c + 1))
                    nc.tensor.matmul(
                        ps_cs[c], ones_b, bw[:, sl], start=first, stop=last
                    )
                for c in range(NCH):
                    sl = slice(CH * c, CH * (c + 1))
                    nc.tensor.matmul(
                        py0[c], xm0[:, P * k : P * (k + 1)], bw[:, sl],
                        start=first, stop=last,
                    )

            # colsum -> SBUF f32 (broadcast over partitions already)
            cs2 = []
            for c in range(NCH):
                ct = cs2p.tile([P, CH], F32, tag="cs2")
                nc.vector.tensor_copy(out=ct, in_=ps_cs[c])
                cs2.append(ct)

            stats_chunk(0, xr0)

            # ---- epilogue helper ---------------------------------------
            def epilogue(m, pys):
                for c in range(NCH):
                    ysb = ypool.tile([P, CH], F32, tag="y")
                    nc.scalar.mul(ysb, pys[c], rbb_c[:, m : m + 1])
                    yo = youtp.tile([P, CH], F32, tag="yo")
                    nc.vector.scalar_tensor_tensor(
                        out=yo, in0=cs2[c], scalar=q_c[:, m : m + 1], in1=ysb,
                        op0=Alu.mult, op1=Alu.add,
                    )
                    nc.gpsimd.dma_start(
                        y[P * m : P * (m + 1), CH * c : CH * (c + 1)], yo
                    )

            epilogue(0, py0)

            # ---- main loop m = 1..15 -----------------------------------
            xr_pending = {}
            for m in range(1, MT):
                xmt = xmp.tile([P, KT * P], BF16, tag="xm")
                nc.sync.dma_start(xmt, xm_d[P * m : P * (m + 1), :])
                # stats chunks 1..3: DMA xrows early, compute when queued
                if m in (1, 5, 9):
                    cc = (m + 3) // 4
                    tiles = []
                    for s in range(4):
                        xr = xrp.tile([P, D], BF16, tag="xr")
                        nc.sync.dma_start(
                            xr, xr_d[P * (4 * cc + s) : P * (4 * cc + s + 1), :]
                        )
                        tiles.append(xr)
                    xr_pending[cc] = tiles
                pys = [
                    psum.tile([P, CH], F32, tag="ps", name=f"py{m}_{c2}")
                    for c2 in range(NCH)
                ]
                for k in range(KT):
                    first, last = k == 0, k == KT - 1
                    lhs = xmt[:, P * k : P * (k + 1)]
                    for c2 in range(NCH):
                        nc.tensor.matmul(
                            pys[c2], lhs, bwts[k][:, CH * c2 : CH * (c2 + 1)],
                            start=first, stop=last,
                        )
                if m in (2, 6, 10):
                    cc = (m + 2) // 4
                    stats_chunk(cc, xr_pending.pop(cc))
                epilogue(m, pys)

    nc.compile()
    return nc


_NC_CACHE = None


def _get_nc():
    global _NC_CACHE
    if _NC_CACHE is None:
        _NC_CACHE = build_nc()
    return _NC_CACHE


def _prep_in_maps(x, fweight):
    bf16 = ml_dtypes.bfloat16
    x2 = np.ascontiguousarray(x, dtype=np.float32).reshape(N_TOK, D)
    fwt = np.ascontiguousarray(np.asarray(fweight, dtype=np.float32).T)
    in_maps = []
    for c in range(N_CORES):
        xc = x2[c * TOK : (c + 1) * TOK, :]
        xr = np.ascontiguousarray(xc).astype(bf16)
        xmb = np.ascontiguousarray(
            xc.reshape(MT, P, KT, P).transpose(0, 3, 2, 1)
        ).astype(bf16).reshape(MT * P, KT * P)
        in_maps.append({"xm": xmb, "xr": xr, "fwt": fwt})
    return in_maps


def run_spmd(x, fweight, **kw):
    nc = _get_nc()
    in_maps = _prep_in_maps(x, fweight)
    return run_bass_kernel_spmd(nc, in_maps, core_ids=list(range(N_CORES)), **kw)


def kernel(x, fweight):
    res = run_spmd(x, fweight)
    y = np.concatenate([res.results[c]["y"] for c in range(N_CORES)], axis=0)
    return y.reshape(4, 4096, O)


if __name__ == "__main__":
    xx = np.random.randn(4, 4096, D).astype(np.float32)
    ww = np.random.uniform(-1 / np.sqrt(D), 1 / np.sqrt(D), (O, D)).astype(np.float32)
    out = kernel(xx, ww)
    print("out", out.shape, out.dtype, float(np.abs(out).mean()))
